# revision 8
# baseline (speedup 1.0000x reference)
"""Trainium2 Bass kernel for nn_Attention_60833916781258 (GAT-style complex attention).

Reference computation (B=2, N=4096, F=128, U=64):
    X_re = H_re @ W ; X_im = H_im @ W
    s = X @ a_1 ; n = X @ a_2 (per re/im)
    E = leaky_relu(s_i + n_j, 0.2)
    alpha1 = softmax(E_re + NEG_BIG*(1-A)) ; alpha2 = softmax(E_im)
    out_re = alpha1 @ X_re - alpha2 @ X_im ; out_im = alpha1 @ X_im + alpha2 @ X_re

Sharding: 8 cores; core c handles batch b=c//4, query-row block rb=c%4 (1024 rows).
Attention tiles live in key-major [j, i] layout on chip.

v3 design:
  - flip-orientation contraction: for each key chunk the attention tile's
    128-query blocks are the matmul's stationary weights and [Xcat | 1] is the
    moving tensor (FD=129). Output lands as [query, channel] in PSUM with the
    softmax denominator as channel 128 -- row-sum matmuls and all epilogue
    transposes disappear. Probe-measured: 91 ns per FD=129 matmul,
    back-to-back, weight loads hidden.
  - 16 accumulators (8 query blocks x {u,v}) are hand-packed 3-per-PSUM-bank
    as 129-column slices of [128,512] bank tiles; the first slice written in
    each bank uses start=True (marks the whole 2KB zero-region), later slices
    rely on the pending-zero init with start=False.
  - im path uses exp(lrelu(s+n)) = max(exp(s)exp(n), exp(.2s)exp(.2n)):
    one fused DVE op (VMAX2) per chunk replaces lrelu+exp; a tunable subset of
    chunk-pairs instead goes through ScalarE (prelu+exp) to balance engines.
  - re path keeps mask-additive LRELU3 on DVE + exp on ScalarE (the mask
    tensor forces a third input stream, which only LRELU3 fuses).
  - everything runs fp16 (PE at 1 cyc/row; fp32 would be 2 instructions at
    4 cyc/row); exp folds a 1/2 scale in via bias to keep f16 headroom.
"""

import sys

if "/opt/trn_rl_repo" not in sys.path:
    sys.path.insert(0, "/opt/trn_rl_repo")

import numpy as np

import concourse.bass as bass
import concourse.tile as tile
from concourse import bacc, mybir
from concourse.bass_utils import run_bass_kernel_spmd

B, N, F, U = 2, 4096, 128, 64
NCORES = 8
ROWS = N * B // NCORES  # 1024 query rows per core
NCHUNK = N // 128  # 32 key chunks of 128
NPAIR = NCHUNK // 2
NBLK = ROWS // 128  # 8 query blocks per core
SLOPE = 0.2
# chunk-pairs whose im-path runs on ScalarE (prelu+exp) instead of DVE VMAX2
ACT_IM_PAIRS = set(range(1, 16, 3))  # 5 pairs -> 10 chunks
MASKV = 448.0  # additive mask magnitude; exp underflows to exactly 0
MLN2 = -0.6931471805599453  # exp bias: weights scaled by 1/2 for f16 headroom
A_ = mybir.AluOpType
AF = mybir.ActivationFunctionType
f32 = mybir.dt.float32
f16 = mybir.dt.float16

# ---------------------------------------------------------------- custom DVE ops


def _register_ops():
    """Register the fused ops in dve_ops.OPS (idempotent)."""
    import concourse.dve_ops as dve_ops_mod
    from concourse.dve_ops import OPS, DveOp, _CUSTOM_DVE_ROW_BASE, _SUB_OPCODE_FOR_NAME
    from concourse.dve_spec import Spec, Src0, Src1, C0, C1, maxx, lower, _has_src1
    from concourse.dve_uop import DveOpSpec

    existing = {op.name: op for op in OPS}
    if "VMAX2_ANT" in existing:
        return (
            existing["LRELU_ADD3_ANT"],
            existing["VMAX2_ANT"],
        )

    def make(name, spec):
        row = _CUSTOM_DVE_ROW_BASE + len(OPS)
        _SUB_OPCODE_FOR_NAME[name] = row
        shas = {}
        for ver in ("v3", "v4"):
            tmp = DveOpSpec(
                name=name, opcode=row, uops=lower(spec, ver=ver), rd1_en=_has_src1(spec)
            )
            shas[ver] = tmp.sha(ver)
        op = DveOp(name, spec, subdim=False, uops_sha=shas)
        OPS.append(op)
        dve_ops_mod.CUSTOM_DVE_SPECS[name] = spec
        return op

    # out = lrelu(in0 + in1 + s0) with slope s1:  t = Src0+Src1+C0; max(t, t*C1)
    t3 = (Src0 + Src1) + C0
    spec3 = Spec(
        body=maxx(t3, t3 * C1),
        reference=lambda in0, in1, s0, s1, imm2: np.maximum(
            (in0 + in1 + s0), (in0 + in1 + s0) * s1
        ),
    )
    lrelu3 = make("LRELU_ADD3_ANT", spec3)
    # out = max(in0*s0, in1*s1)   (separable exp(lrelu(s+n)) for the im part)
    spec_vmax = Spec(
        body=maxx(Src0 * C0, Src1 * C1),
        reference=lambda in0, in1, s0, s1, imm2: np.maximum(in0 * s0, in1 * s1),
    )
    vmax2 = make("VMAX2_ANT", spec_vmax)
    return lrelu3, vmax2


# ---------------------------------------------------------------- device program

_PROGRAM_CACHE = {}

# psum slot map: (comp, ib) -> (bank_tile, slot) with 3 slots of 129 per bank
def _slot(comp, ib):
    idx = comp * NBLK + ib  # 0..15
    return idx // 3, idx % 3


def _build_program():
    if "nc" in _PROGRAM_CACHE:
        return _PROGRAM_CACHE["nc"]
    LRELU3, VMAX2 = _register_ops()

    nc = bacc.Bacc("TRN2", target_bir_lowering=False, debug=False, num_devices=NCORES)
    from concourse.tile_rust import add_dep_helper

    _pe_prev = [None]

    def mm(out, lhsT, rhs, reuse=False, **kw):
        bi = nc.tensor.matmul(out, lhsT=lhsT, rhs=rhs, **kw)
        if reuse:
            bi.ins.ldweights = False
        if _pe_prev[0] is not None:
            add_dep_helper(bi.ins, _pe_prev[0], sync=False, reason="pe order")
        _pe_prev[0] = bi.ins
        return bi

    dp = nc.dram_tensor
    ht_in = {
        "re": dp("ht_re", [F, N], f16, kind="ExternalInput").ap(),
        "im": dp("ht_im", [F, N], f16, kind="ExternalInput").ap(),
    }
    w_in = dp("w", [F, U], f16, kind="ExternalInput").ap()
    wt_in = dp("wt", [U, F], f16, kind="ExternalInput").ap()
    acat_in = dp("acat", [U, 2], f16, kind="ExternalInput").ap()
    mt_in = dp("mt", [N, ROWS], f16, kind="ExternalInput").ap()
    o_re = dp("o_re", [ROWS, U], f32, kind="ExternalOutput").ap()
    o_im = dp("o_im", [ROWS, U], f32, kind="ExternalOutput").ap()

    with tile.TileContext(nc) as tc:
        with tc.tile_pool(name="cst", bufs=1) as cst:
            # ---- small constants
            w_sb = cst.tile([F, U], f16, tag="w", name="w")
            nc.sync.dma_start(w_sb[:], w_in[:])
            wt_sb = cst.tile([U, F], f16, tag="wt", name="wt")
            nc.sync.dma_start(wt_sb[:], wt_in[:])
            acat_sb = cst.tile([U, 2], f16, tag="acat", name="acat")
            nc.sync.dma_start(acat_sb[:], acat_in[:])
            mln2_sb = cst.tile([128, 1], f32, tag="mln2", name="mln2")
            nc.gpsimd.memset(mln2_sb[:], MLN2)

            # ---- ht loaded in full up front (2 MB total)
            ht_sb = {}
            for nm in ("re", "im"):
                t = cst.tile([F, N], f16, tag=f"ht_{nm}", name=f"ht_{nm}")
                ht_sb[nm] = t
                nc.gpsimd.dma_start(t[:], ht_in[nm][:])

            # [Xre | Xim | 1 | pad] per chunk, 130 columns (128..129 preset to 1)
            xcp_sb = cst.tile([128, NCHUNK * 130], f16, tag="xcp", name="xcp")
            nc.gpsimd.memset(xcp_sb[:], 1.0)
            xcp3 = xcp_sb[:].rearrange("p (c u) -> p c u", u=130)
            sn_sb = {
                nm: cst.tile([128, 2 * NCHUNK], f32, tag=f"sn_{nm}", name=f"sn_{nm}")
                for nm in ("re", "im")
            }
            sbc_sb = {
                nm: cst.tile([128, ROWS], f16, tag=f"sbc_{nm}", name=f"sbc_{nm}")
                for nm in ("re", "im")
            }
            # exp(n_im)/1 and exp(.2 n_im) per key, chunk-major [128, NCHUNK]
            c_sb = cst.tile([128, NCHUNK], f32, tag="c", name="c")
            d_sb = cst.tile([128, NCHUNK], f32, tag="d", name="d")
            # exp(s_im)/2, exp(.2 s_im)/2 broadcast over partitions [128, ROWS]
            pb_sb = cst.tile([128, ROWS], f16, tag="pb", name="pb")
            qb_sb = cst.tile([128, ROWS], f16, tag="qb", name="qb")

            # ---- phase b: wa, then s-broadcast in one matmul hop via
            #      W1[F, p] = wa[F, 0] (rank-1 weights -> out[p, i] = s_i for all p)
            ones128_sb = cst.tile([128, 128], f16, tag="ones128", name="ones128")
            nc.gpsimd.memset(ones128_sb[:], 1.0)
            with tc.tile_pool(name="psB", bufs=1, space="PSUM") as psB:
                wa_ps = psB.tile([F, 2], f32, tag="wa_ps", name="wa_ps")
                mm(wa_ps[:], wt_sb[:], acat_sb[:], start=True, stop=True)
                wa16_sb = cst.tile([F, 2], f16, tag="wa16", name="wa16")
                nc.vector.tensor_copy(wa16_sb[:], wa_ps[:])
                w1_sb = cst.tile([128, 128], f16, tag="w1b", name="w1b")
                nc.vector.tensor_scalar_mul(w1_sb[:], ones128_sb[:], wa_ps[:, 0:1])
                for nm in ("re", "im"):
                    sb_ps = psB.tile([128, ROWS], f32, tag="sb_ps", name="sb_ps")
                    for h in range(2):
                        mm(
                            sb_ps[:, 512 * h : 512 * (h + 1)],
                            w1_sb[:],
                            ht_sb[nm][:, 512 * h : 512 * (h + 1)],
                            start=True,
                            stop=True,
                        )
                    nc.scalar.copy(sbc_sb[nm][:], sb_ps[:])
                # pb/qb from the already-broadcast sbc_im (1/2 scale folded in)
                nc.scalar.activation(pb_sb[:], sbc_sb["im"][:], AF.Exp, bias=mln2_sb[:])
                nc.scalar.activation(
                    qb_sb[:], sbc_sb["im"][:], AF.Exp, bias=mln2_sb[:], scale=SLOPE
                )

            # ---- interleaved groups: X/sn generation + attention chunks
            with (
                tc.tile_pool(name="psG", bufs=1, space="PSUM") as psG,
                tc.tile_pool(name="psM", bufs=1, space="PSUM") as psM,
                tc.tile_pool(name="mtp", bufs=6) as mt_pool,
                tc.tile_pool(name="rp", bufs=3) as r_pool,
                tc.tile_pool(name="uvp", bufs=3) as uv_pool,
            ):
                # 6 PSUM bank tiles, each holding 3 slices of 129 columns
                pbank = [
                    psM.tile([128, 512], f32, tag=f"pb{i}", name=f"pbank{i}")
                    for i in range(6)
                ]

                def pslot(comp, ib):
                    bt, sl = _slot(comp, ib)
                    return pbank[bt][:, 129 * sl : 129 * sl + 129]

                started_banks = set()

                # ---- X + s/n generation for all 32 chunks up front
                for g in range(4):
                    for nm in ("re", "im"):
                        sn_ps = psG.tile([128, 16], f32, tag="sn_ps", name="sn_ps")
                        x_ps = psG.tile([128, 512], f32, tag="x_ps", name="x_ps")
                        for m in range(8):
                            k = 8 * g + m
                            hchunk = ht_sb[nm][:, 128 * k : 128 * (k + 1)]
                            mm(
                                x_ps[:, 64 * m : 64 * (m + 1)],
                                hchunk,
                                w_sb[:],
                                start=True,
                                stop=True,
                            )
                            mm(
                                sn_ps[:, 2 * m : 2 * m + 2],
                                hchunk,
                                wa16_sb[:],
                                start=True,
                                stop=True,
                                reuse=True,
                            )
                        off = 0 if nm == "re" else 64
                        nc.scalar.copy(
                            xcp3[:, 8 * g : 8 * g + 8, off : off + 64],
                            x_ps[:].rearrange("p (c u) -> p c u", u=64),
                        )
                        nc.vector.tensor_copy(
                            sn_sb[nm][:, 16 * g : 16 * g + 16], sn_ps[:]
                        )
                nim = sn_sb["im"][:].rearrange("p (c two) -> p c two", two=2)
                nc.scalar.activation(c_sb[:], nim[:, :, 1], AF.Exp)
                nc.scalar.activation(d_sb[:], nim[:, :, 1], AF.Exp, scale=SLOPE)

                # ---- attention pairs, free-running pipeline
                if True:
                    for pr in range(NPAIR):
                        ka = 2 * pr
                        act_im = pr in ACT_IM_PAIRS
                        rw = r_pool.tile([128, 4 * ROWS], f16, tag="rw", name="rw")
                        uvw = uv_pool.tile([128, 4 * ROWS], f16, tag="uv", name="uvw")
                        for c2 in range(2):
                            k = ka + c2
                            mt_t = mt_pool.tile([128, ROWS], f16, tag="mt", name="mt")
                            nc.sync.dma_start(
                                mt_t[:], mt_in[128 * k : 128 * (k + 1), :]
                            )
                            nc.vector._custom_dve(
                                LRELU3,
                                out=rw[:, ROWS * c2 : ROWS * (c2 + 1)],
                                in0=mt_t[:],
                                in1=sbc_sb["re"][:],
                                s0=sn_sb["re"][:, 2 * k + 1 : 2 * k + 2],
                                s1=SLOPE,
                            )
                            if act_im:
                                nc.scalar.activation(
                                    rw[:, ROWS * (2 + c2) : ROWS * (3 + c2)],
                                    sbc_sb["im"][:],
                                    AF.Prelu,
                                    bias=sn_sb["im"][:, 2 * k + 1 : 2 * k + 2],
                                    alpha=SLOPE,
                                )
                            else:
                                nc.vector._custom_dve(
                                    VMAX2,
                                    out=uvw[:, ROWS * (2 + c2) : ROWS * (3 + c2)],
                                    in0=pb_sb[:],
                                    in1=qb_sb[:],
                                    s0=c_sb[:, k : k + 1],
                                    s1=d_sb[:, k : k + 1],
                                )
                        if act_im:
                            # one wide exp covers re pair + im pair
                            nc.scalar.activation(
                                uvw[:, 0 : 4 * ROWS], rw[:, 0 : 4 * ROWS], AF.Exp,
                                bias=mln2_sb[:],
                            )
                        else:
                            nc.scalar.activation(
                                uvw[:, 0 : 2 * ROWS], rw[:, 0 : 2 * ROWS], AF.Exp,
                                bias=mln2_sb[:],
                            )
                        # flip-orientation matmuls: attention blocks are the
                        # stationary weights; [Xcat | 1] streams through.
                        # (comp, ib) emitted slot-major so consecutive matmuls
                        # hit different PSUM banks and slot-0 starts come first.
                        order = sorted(
                            ((comp, ib) for comp in range(2) for ib in range(NBLK)),
                            key=lambda ci: (_slot(ci[0], ci[1])[1], _slot(ci[0], ci[1])[0]),
                        )
                        for c2 in range(2):
                            k = ka + c2
                            rhs = xcp3[:, k, 0:129]
                            for comp, ib in order:
                                base = ROWS * (2 * comp + c2)
                                out_ap = pslot(comp, ib)
                                bt, sl = _slot(comp, ib)
                                if k == 0:
                                    st = bt not in started_banks
                                    started_banks.add(bt)
                                else:
                                    st = False
                                mm(
                                    out_ap,
                                    uvw[:, base + 128 * ib : base + 128 * (ib + 1)],
                                    rhs,
                                    start=st,
                                    stop=(k == NCHUNK - 1),
                                    skip_group_check=True,
                                )

                # ---- epilogue: normalize and combine directly from PSUM
                with tc.tile_pool(name="ep2", bufs=8) as ep2:
                    for ib in range(NBLK):
                        pu = pslot(0, ib)
                        pv = pslot(1, ib)
                        rr = ep2.tile([128, 2], f32, tag="rr", name="rr")
                        nc.vector.reciprocal(rr[:, 0:1], pu[:, 128:129])
                        nc.vector.reciprocal(rr[:, 1:2], pv[:, 128:129])
                        sl = slice(128 * ib, 128 * (ib + 1))
                        # out_re = pu[:, :64]/su - pv[:, 64:]/sv
                        w1 = ep2.tile([128, U], f32, tag="w1", name="w1")
                        nc.scalar.activation(
                            w1[:], pv[:, U : 2 * U], AF.Copy, scale=rr[:, 1:2]
                        )
                        ore_sb = ep2.tile([128, U], f32, tag="ore", name="ore")
                        nc.vector.scalar_tensor_tensor(
                            ore_sb[:],
                            pu[:, 0:U],
                            rr[:, 0:1],
                            w1[:],
                            A_.mult,
                            A_.subtract,
                        )
                        nc.sync.dma_start(o_re[sl, :], ore_sb[:])
                        # out_im = pu[:, 64:]/su + pv[:, :64]/sv
                        w2 = ep2.tile([128, U], f32, tag="w2", name="w2")
                        nc.scalar.activation(
                            w2[:], pv[:, 0:U], AF.Copy, scale=rr[:, 1:2]
                        )
                        oim_sb = ep2.tile([128, U], f32, tag="oim", name="oim")
                        nc.vector.scalar_tensor_tensor(
                            oim_sb[:],
                            pu[:, U : 2 * U],
                            rr[:, 0:1],
                            w2[:],
                            A_.mult,
                            A_.add,
                        )
                        nc.sync.dma_start(o_im[sl, :], oim_sb[:])

    nc.compile()
    _PROGRAM_CACHE["nc"] = nc
    return nc


# ---------------------------------------------------------------- host wrapper


def _make_in_maps(H_re, H_im, A, W, a_1, a_2):
    W16 = np.asarray(W, np.float16)
    acat = np.concatenate(
        [np.asarray(a_1, np.float16), np.asarray(a_2, np.float16)], axis=1
    )
    shared = {
        "w": W16,
        "wt": np.ascontiguousarray(W16.T),
        "acat": acat,
    }
    in_maps = []
    for c in range(NCORES):
        b, rb = divmod(c, NCORES // B)
        r0 = rb * ROWS
        hre = np.asarray(H_re[b], np.float16)
        him = np.asarray(H_im[b], np.float16)
        ab = np.asarray(A[b], np.float32)
        # key order rolled so this core's own query rows come first
        mt = np.ascontiguousarray(
            ((np.roll(ab[r0 : r0 + ROWS].T, -r0, axis=0) - 1.0) * MASKV).astype(
                np.float16
            )
        )
        in_maps.append(
            {
                **shared,
                "ht_re": np.ascontiguousarray(np.roll(hre, -r0, axis=0).T),
                "ht_im": np.ascontiguousarray(np.roll(him, -r0, axis=0).T),
                "mt": mt,
            }
        )
    return in_maps


def kernel(H_re, H_im, A, W, a_1, a_2):
    nc = _build_program()
    in_maps = _make_in_maps(H_re, H_im, A, W, a_1, a_2)
    res = run_bass_kernel_spmd(nc, in_maps, list(range(NCORES)))
    out_re = np.empty((B, N, U), np.float32)
    out_im = np.empty((B, N, U), np.float32)
    for c in range(NCORES):
        b, rb = divmod(c, NCORES // B)
        r0 = rb * ROWS
        out_re[b, r0 : r0 + ROWS] = res.results[c]["o_re"]
        out_im[b, r0 : r0 + ROWS] = res.results[c]["o_im"]
    return out_re, out_im


# revision 10
# speedup vs baseline: 1.2206x; 1.2206x over previous
"""Trainium2 Bass kernel for nn_Attention_60833916781258 (GAT-style complex attention).

Reference computation (B=2, N=4096, F=128, U=64):
    X_re = H_re @ W ; X_im = H_im @ W
    s = X @ a_1 ; n = X @ a_2 (per re/im)
    E = leaky_relu(s_i + n_j, 0.2)
    alpha1 = softmax(E_re + NEG_BIG*(1-A)) ; alpha2 = softmax(E_im)
    out_re = alpha1 @ X_re - alpha2 @ X_im ; out_im = alpha1 @ X_im + alpha2 @ X_re

Sharding: 8 cores; core c handles batch b=c//4, query-row block rb=c%4 (1024 rows).
Attention tiles live in key-major [j, i] layout on chip.

v3 design:
  - flip-orientation contraction: for each key chunk the attention tile's
    128-query blocks are the matmul's stationary weights and [Xcat | 1] is the
    moving tensor (FD=129). Output lands as [query, channel] in PSUM with the
    softmax denominator as channel 128 -- row-sum matmuls and all epilogue
    transposes disappear. Probe-measured: 91 ns per FD=129 matmul,
    back-to-back, weight loads hidden.
  - 16 accumulators (8 query blocks x {u,v}) are hand-packed 3-per-PSUM-bank
    as 129-column slices of [128,512] bank tiles; the first slice written in
    each bank uses start=True (marks the whole 2KB zero-region), later slices
    rely on the pending-zero init with start=False.
  - im path uses exp(lrelu(s+n)) = max(exp(s)exp(n), exp(.2s)exp(.2n)):
    one fused DVE op (VMAX2) per chunk replaces lrelu+exp; a tunable subset of
    chunk-pairs instead goes through ScalarE (prelu+exp) to balance engines.
  - re path keeps mask-additive LRELU3 on DVE + exp on ScalarE (the mask
    tensor forces a third input stream, which only LRELU3 fuses).
  - everything runs fp16 (PE at 1 cyc/row; fp32 would be 2 instructions at
    4 cyc/row); exp folds a 1/2 scale in via bias to keep f16 headroom.
"""

import sys

if "/opt/trn_rl_repo" not in sys.path:
    sys.path.insert(0, "/opt/trn_rl_repo")

import numpy as np

import concourse.bass as bass
import concourse.tile as tile
from concourse import bacc, mybir
from concourse.bass_utils import run_bass_kernel_spmd

B, N, F, U = 2, 4096, 128, 64
NCORES = 8
ROWS = N * B // NCORES  # 1024 query rows per core
NCHUNK = N // 128  # 32 key chunks of 128
NPAIR = NCHUNK // 2
NBLK = ROWS // 128  # 8 query blocks per core
SLOPE = 0.2
# chunk-pairs whose im-path runs on ScalarE (prelu+exp) instead of DVE VMAX2
ACT_IM_PAIRS = set(range(1, 16, 3))  # 5 pairs -> 10 chunks
# chunk-pairs whose im-path max runs on GpSimd (DVE makes the two products)
POOL_IM_PAIRS = set()  # gpsimd tensor ops rejected by walrus codegen
MASKV = 448.0  # additive mask magnitude; exp underflows to exactly 0
MLN2 = -0.6931471805599453  # exp bias: weights scaled by 1/2 for f16 headroom
A_ = mybir.AluOpType
AF = mybir.ActivationFunctionType
f32 = mybir.dt.float32
f16 = mybir.dt.float16

# ---------------------------------------------------------------- custom DVE ops


def _register_ops():
    """Register the fused ops in dve_ops.OPS (idempotent)."""
    import concourse.dve_ops as dve_ops_mod
    from concourse.dve_ops import OPS, DveOp, _CUSTOM_DVE_ROW_BASE, _SUB_OPCODE_FOR_NAME
    from concourse.dve_spec import Spec, Src0, Src1, C0, C1, maxx, lower, _has_src1
    from concourse.dve_uop import DveOpSpec

    existing = {op.name: op for op in OPS}
    if "VMAX2_ANT" in existing:
        return (
            existing["LRELU_ADD3_ANT"],
            existing["VMAX2_ANT"],
        )

    def make(name, spec):
        row = _CUSTOM_DVE_ROW_BASE + len(OPS)
        _SUB_OPCODE_FOR_NAME[name] = row
        shas = {}
        for ver in ("v3", "v4"):
            tmp = DveOpSpec(
                name=name, opcode=row, uops=lower(spec, ver=ver), rd1_en=_has_src1(spec)
            )
            shas[ver] = tmp.sha(ver)
        op = DveOp(name, spec, subdim=False, uops_sha=shas)
        OPS.append(op)
        dve_ops_mod.CUSTOM_DVE_SPECS[name] = spec
        return op

    # out = lrelu(in0 + in1 + s0) with slope s1:  t = Src0+Src1+C0; max(t, t*C1)
    t3 = (Src0 + Src1) + C0
    spec3 = Spec(
        body=maxx(t3, t3 * C1),
        reference=lambda in0, in1, s0, s1, imm2: np.maximum(
            (in0 + in1 + s0), (in0 + in1 + s0) * s1
        ),
    )
    lrelu3 = make("LRELU_ADD3_ANT", spec3)
    # out = max(in0*s0, in1*s1)   (separable exp(lrelu(s+n)) for the im part)
    spec_vmax = Spec(
        body=maxx(Src0 * C0, Src1 * C1),
        reference=lambda in0, in1, s0, s1, imm2: np.maximum(in0 * s0, in1 * s1),
    )
    vmax2 = make("VMAX2_ANT", spec_vmax)
    return lrelu3, vmax2


# ---------------------------------------------------------------- device program

_PROGRAM_CACHE = {}

# psum slot map: (comp, ib) -> (bank_tile, slot) with 3 slots of 129 per bank
def _slot(comp, ib):
    idx = comp * NBLK + ib  # 0..15
    return idx // 3, idx % 3


def _build_program():
    if "nc" in _PROGRAM_CACHE:
        return _PROGRAM_CACHE["nc"]
    LRELU3, VMAX2 = _register_ops()

    nc = bacc.Bacc("TRN2", target_bir_lowering=False, debug=False, num_devices=NCORES)
    from concourse.tile_rust import add_dep_helper

    _pe_prev = [None]

    def mm(out, lhsT, rhs, reuse=False, **kw):
        bi = nc.tensor.matmul(out, lhsT=lhsT, rhs=rhs, **kw)
        if reuse:
            bi.ins.ldweights = False
        if _pe_prev[0] is not None:
            add_dep_helper(bi.ins, _pe_prev[0], sync=False, reason="pe order")
        _pe_prev[0] = bi.ins
        return bi

    dp = nc.dram_tensor
    ht_in = {
        "re": dp("ht_re", [F, N], f16, kind="ExternalInput").ap(),
        "im": dp("ht_im", [F, N], f16, kind="ExternalInput").ap(),
    }
    w_in = dp("w", [F, U], f16, kind="ExternalInput").ap()
    wt_in = dp("wt", [U, F], f16, kind="ExternalInput").ap()
    acat_in = dp("acat", [U, 2], f16, kind="ExternalInput").ap()
    mt_in = dp("mt", [N, ROWS], f16, kind="ExternalInput").ap()
    o_re = dp("o_re", [ROWS, U], f32, kind="ExternalOutput").ap()
    o_im = dp("o_im", [ROWS, U], f32, kind="ExternalOutput").ap()

    with tile.TileContext(nc) as tc:
        with tc.tile_pool(name="cst", bufs=1) as cst:
            # ---- small constants
            w_sb = cst.tile([F, U], f16, tag="w", name="w")
            nc.sync.dma_start(w_sb[:], w_in[:])
            wt_sb = cst.tile([U, F], f16, tag="wt", name="wt")
            nc.sync.dma_start(wt_sb[:], wt_in[:])
            acat_sb = cst.tile([U, 2], f16, tag="acat", name="acat")
            nc.sync.dma_start(acat_sb[:], acat_in[:])
            mln2_sb = cst.tile([128, 1], f32, tag="mln2", name="mln2")
            nc.gpsimd.memset(mln2_sb[:], MLN2)

            # ---- ht loaded up front in group-sized pieces (overlaps X-gen)
            ht_sb = {}
            for nm in ("re", "im"):
                t = cst.tile([F, N], f16, tag=f"ht_{nm}", name=f"ht_{nm}")
                ht_sb[nm] = t
            for g in range(4):
                for nm in ("re", "im"):
                    lo = ROWS * g
                    nc.gpsimd.dma_start(
                        ht_sb[nm][:, lo : lo + ROWS], ht_in[nm][:, lo : lo + ROWS]
                    )

            # [Xre | Xim | 1 | pad] per chunk, 130 columns (128..129 preset to 1)
            xcp_sb = cst.tile([128, NCHUNK * 130], f16, tag="xcp", name="xcp")
            nc.gpsimd.memset(xcp_sb[:], 1.0)
            xcp3 = xcp_sb[:].rearrange("p (c u) -> p c u", u=130)
            sn_sb = {
                nm: cst.tile([128, 2 * NCHUNK], f32, tag=f"sn_{nm}", name=f"sn_{nm}")
                for nm in ("re", "im")
            }
            sbc_sb = {
                nm: cst.tile([128, ROWS], f16, tag=f"sbc_{nm}", name=f"sbc_{nm}")
                for nm in ("re", "im")
            }
            # exp(n_im)/1 and exp(.2 n_im) per key, chunk-major [128, NCHUNK]
            c_sb = cst.tile([128, NCHUNK], f32, tag="c", name="c")
            d_sb = cst.tile([128, NCHUNK], f32, tag="d", name="d")
            # exp(s_im)/2, exp(.2 s_im)/2 broadcast over partitions [128, ROWS]
            pb_sb = cst.tile([128, ROWS], f16, tag="pb", name="pb")
            qb_sb = cst.tile([128, ROWS], f16, tag="qb", name="qb")

            # ---- phase b: wa, then s-broadcast in one matmul hop via
            #      W1[F, p] = wa[F, 0] (rank-1 weights -> out[p, i] = s_i for all p)
            ones128_sb = cst.tile([128, 128], f16, tag="ones128", name="ones128")
            nc.gpsimd.memset(ones128_sb[:], 1.0)
            with tc.tile_pool(name="psB", bufs=1, space="PSUM") as psB:
                wa_ps = psB.tile([F, 2], f32, tag="wa_ps", name="wa_ps")
                mm(wa_ps[:], wt_sb[:], acat_sb[:], start=True, stop=True)
                wa16_sb = cst.tile([F, 2], f16, tag="wa16", name="wa16")
                nc.vector.tensor_copy(wa16_sb[:], wa_ps[:])
                w1_sb = cst.tile([128, 128], f16, tag="w1b", name="w1b")
                nc.vector.tensor_scalar_mul(w1_sb[:], ones128_sb[:], wa_ps[:, 0:1])
                for nm in ("re", "im"):
                    sb_ps = psB.tile([128, ROWS], f32, tag="sb_ps", name="sb_ps")
                    for h in range(2):
                        mm(
                            sb_ps[:, 512 * h : 512 * (h + 1)],
                            w1_sb[:],
                            ht_sb[nm][:, 512 * h : 512 * (h + 1)],
                            start=True,
                            stop=True,
                        )
                    nc.scalar.copy(sbc_sb[nm][:], sb_ps[:])
                # pb/qb from the already-broadcast sbc_im (1/2 scale folded in)
                nc.scalar.activation(pb_sb[:], sbc_sb["im"][:], AF.Exp, bias=mln2_sb[:])
                nc.scalar.activation(
                    qb_sb[:], sbc_sb["im"][:], AF.Exp, bias=mln2_sb[:], scale=SLOPE
                )

            # ---- interleaved groups: X/sn generation + attention chunks
            with (
                tc.tile_pool(name="psG", bufs=1, space="PSUM") as psG,
                tc.tile_pool(name="psM", bufs=1, space="PSUM") as psM,
                tc.tile_pool(name="mtp", bufs=6) as mt_pool,
                tc.tile_pool(name="rp", bufs=3) as r_pool,
                tc.tile_pool(name="uvp", bufs=3) as uv_pool,
                tc.tile_pool(name="tscp", bufs=3) as tsc_pool,
            ):
                # 6 PSUM bank tiles, each holding 3 slices of 129 columns
                pbank = [
                    psM.tile([128, 512], f32, tag=f"pb{i}", name=f"pbank{i}")
                    for i in range(6)
                ]

                def pslot(comp, ib):
                    bt, sl = _slot(comp, ib)
                    return pbank[bt][:, 129 * sl : 129 * sl + 129]

                started_banks = set()
                nim = sn_sb["im"][:].rearrange("p (c two) -> p c two", two=2)

                def xgen(g):
                    """X/s/n generation + c/d for group g (8 chunks)."""
                    for nm in ("re", "im"):
                        sn_ps = psG.tile([128, 16], f32, tag="sn_ps", name="sn_ps")
                        x_ps = psG.tile([128, 512], f32, tag="x_ps", name="x_ps")
                        for m in range(8):
                            k = 8 * g + m
                            hchunk = ht_sb[nm][:, 128 * k : 128 * (k + 1)]
                            mm(
                                x_ps[:, 64 * m : 64 * (m + 1)],
                                hchunk,
                                w_sb[:],
                                start=True,
                                stop=True,
                            )
                            mm(
                                sn_ps[:, 2 * m : 2 * m + 2],
                                hchunk,
                                wa16_sb[:],
                                start=True,
                                stop=True,
                                reuse=True,
                            )
                        off = 0 if nm == "re" else 64
                        nc.scalar.copy(
                            xcp3[:, 8 * g : 8 * g + 8, off : off + 64],
                            x_ps[:].rearrange("p (c u) -> p c u", u=64),
                        )
                        nc.vector.tensor_copy(
                            sn_sb[nm][:, 16 * g : 16 * g + 16], sn_ps[:]
                        )
                    nc.scalar.activation(
                        c_sb[:, 8 * g : 8 * g + 8], nim[:, 8 * g : 8 * g + 8, 1],
                        AF.Exp,
                    )
                    nc.scalar.activation(
                        d_sb[:, 8 * g : 8 * g + 8], nim[:, 8 * g : 8 * g + 8, 1],
                        AF.Exp, scale=SLOPE,
                    )

                xgen(0)
                xgen(1)

                # ---- attention pairs, with group g+2's X-gen interleaved
                for g in range(4):
                    if g + 2 < 4:
                        xgen(g + 2)
                    for t2i in range(4):
                        pr = 4 * g + t2i
                        ka = 2 * pr
                        act_im = pr in ACT_IM_PAIRS
                        rw = r_pool.tile([128, 4 * ROWS], f16, tag="rw", name="rw")
                        uvw = uv_pool.tile([128, 4 * ROWS], f16, tag="uv", name="uvw")
                        for c2 in range(2):
                            k = ka + c2
                            mt_t = mt_pool.tile([128, ROWS], f16, tag="mt", name="mt")
                            nc.sync.dma_start(
                                mt_t[:], mt_in[128 * k : 128 * (k + 1), :]
                            )
                            nc.vector._custom_dve(
                                LRELU3,
                                out=rw[:, ROWS * c2 : ROWS * (c2 + 1)],
                                in0=mt_t[:],
                                in1=sbc_sb["re"][:],
                                s0=sn_sb["re"][:, 2 * k + 1 : 2 * k + 2],
                                s1=SLOPE,
                            )
                            if act_im:
                                nc.scalar.activation(
                                    rw[:, ROWS * (2 + c2) : ROWS * (3 + c2)],
                                    sbc_sb["im"][:],
                                    AF.Prelu,
                                    bias=sn_sb["im"][:, 2 * k + 1 : 2 * k + 2],
                                    alpha=SLOPE,
                                )
                            elif pr in POOL_IM_PAIRS:
                                t1 = tsc_pool.tile(
                                    [128, ROWS], f16, tag="t1", name="t1"
                                )
                                t2 = tsc_pool.tile(
                                    [128, ROWS], f16, tag="t2", name="t2"
                                )
                                nc.vector.tensor_scalar(
                                    t1[:], pb_sb[:], c_sb[:, k : k + 1], None,
                                    A_.mult,
                                )
                                nc.vector.tensor_scalar(
                                    t2[:], qb_sb[:], d_sb[:, k : k + 1], None,
                                    A_.mult,
                                )
                                nc.gpsimd.tensor_max(
                                    uvw[:, ROWS * (2 + c2) : ROWS * (3 + c2)],
                                    t1[:],
                                    t2[:],
                                )
                            else:
                                nc.vector._custom_dve(
                                    VMAX2,
                                    out=uvw[:, ROWS * (2 + c2) : ROWS * (3 + c2)],
                                    in0=pb_sb[:],
                                    in1=qb_sb[:],
                                    s0=c_sb[:, k : k + 1],
                                    s1=d_sb[:, k : k + 1],
                                )
                        if act_im:
                            # one wide exp covers re pair + im pair
                            nc.scalar.activation(
                                uvw[:, 0 : 4 * ROWS], rw[:, 0 : 4 * ROWS], AF.Exp,
                                bias=mln2_sb[:],
                            )
                        else:
                            nc.scalar.activation(
                                uvw[:, 0 : 2 * ROWS], rw[:, 0 : 2 * ROWS], AF.Exp,
                                bias=mln2_sb[:],
                            )
                        # flip-orientation matmuls: attention blocks are the
                        # stationary weights; [Xcat | 1] streams through.
                        # (comp, ib) emitted slot-major so consecutive matmuls
                        # hit different PSUM banks and slot-0 starts come first.
                        order = sorted(
                            ((comp, ib) for comp in range(2) for ib in range(NBLK)),
                            key=lambda ci: (_slot(ci[0], ci[1])[1], _slot(ci[0], ci[1])[0]),
                        )
                        for c2 in range(2):
                            k = ka + c2
                            rhs = xcp3[:, k, 0:129]
                            for comp, ib in order:
                                base = ROWS * (2 * comp + c2)
                                out_ap = pslot(comp, ib)
                                bt, sl = _slot(comp, ib)
                                if k == 0:
                                    st = bt not in started_banks
                                    started_banks.add(bt)
                                else:
                                    st = False
                                mm(
                                    out_ap,
                                    uvw[:, base + 128 * ib : base + 128 * (ib + 1)],
                                    rhs,
                                    start=st,
                                    stop=(k == NCHUNK - 1),
                                    skip_group_check=True,
                                )

                # ---- epilogue: normalize and combine directly from PSUM
                with tc.tile_pool(name="ep2", bufs=8) as ep2:
                    for ib in range(NBLK):
                        pu = pslot(0, ib)
                        pv = pslot(1, ib)
                        rr = ep2.tile([128, 2], f32, tag="rr", name="rr")
                        nc.vector.reciprocal(rr[:, 0:1], pu[:, 128:129])
                        nc.vector.reciprocal(rr[:, 1:2], pv[:, 128:129])
                        sl = slice(128 * ib, 128 * (ib + 1))
                        # out_re = pu[:, :64]/su - pv[:, 64:]/sv
                        w1 = ep2.tile([128, U], f32, tag="w1", name="w1")
                        nc.scalar.activation(
                            w1[:], pv[:, U : 2 * U], AF.Copy, scale=rr[:, 1:2]
                        )
                        ore_sb = ep2.tile([128, U], f32, tag="ore", name="ore")
                        nc.vector.scalar_tensor_tensor(
                            ore_sb[:],
                            pu[:, 0:U],
                            rr[:, 0:1],
                            w1[:],
                            A_.mult,
                            A_.subtract,
                        )
                        nc.sync.dma_start(o_re[sl, :], ore_sb[:])
                        # out_im = pu[:, 64:]/su + pv[:, :64]/sv
                        w2 = ep2.tile([128, U], f32, tag="w2", name="w2")
                        nc.scalar.activation(
                            w2[:], pv[:, 0:U], AF.Copy, scale=rr[:, 1:2]
                        )
                        oim_sb = ep2.tile([128, U], f32, tag="oim", name="oim")
                        nc.vector.scalar_tensor_tensor(
                            oim_sb[:],
                            pu[:, U : 2 * U],
                            rr[:, 0:1],
                            w2[:],
                            A_.mult,
                            A_.add,
                        )
                        nc.sync.dma_start(o_im[sl, :], oim_sb[:])

    nc.compile()
    _PROGRAM_CACHE["nc"] = nc
    return nc


# ---------------------------------------------------------------- host wrapper


def _make_in_maps(H_re, H_im, A, W, a_1, a_2):
    W16 = np.asarray(W, np.float16)
    acat = np.concatenate(
        [np.asarray(a_1, np.float16), np.asarray(a_2, np.float16)], axis=1
    )
    shared = {
        "w": W16,
        "wt": np.ascontiguousarray(W16.T),
        "acat": acat,
    }
    in_maps = []
    for c in range(NCORES):
        b, rb = divmod(c, NCORES // B)
        r0 = rb * ROWS
        hre = np.asarray(H_re[b], np.float16)
        him = np.asarray(H_im[b], np.float16)
        ab = np.asarray(A[b], np.float32)
        # key order rolled so this core's own query rows come first
        mt = np.ascontiguousarray(
            ((np.roll(ab[r0 : r0 + ROWS].T, -r0, axis=0) - 1.0) * MASKV).astype(
                np.float16
            )
        )
        in_maps.append(
            {
                **shared,
                "ht_re": np.ascontiguousarray(np.roll(hre, -r0, axis=0).T),
                "ht_im": np.ascontiguousarray(np.roll(him, -r0, axis=0).T),
                "mt": mt,
            }
        )
    return in_maps


def kernel(H_re, H_im, A, W, a_1, a_2):
    nc = _build_program()
    in_maps = _make_in_maps(H_re, H_im, A, W, a_1, a_2)
    res = run_bass_kernel_spmd(nc, in_maps, list(range(NCORES)))
    out_re = np.empty((B, N, U), np.float32)
    out_im = np.empty((B, N, U), np.float32)
    for c in range(NCORES):
        b, rb = divmod(c, NCORES // B)
        r0 = rb * ROWS
        out_re[b, r0 : r0 + ROWS] = res.results[c]["o_re"]
        out_im[b, r0 : r0 + ROWS] = res.results[c]["o_im"]
    return out_re, out_im


# revision 12
# speedup vs baseline: 1.2437x; 1.0189x over previous
"""Trainium2 Bass kernel for nn_Attention_60833916781258 (GAT-style complex attention).

Reference computation (B=2, N=4096, F=128, U=64):
    X_re = H_re @ W ; X_im = H_im @ W
    s = X @ a_1 ; n = X @ a_2 (per re/im)
    E = leaky_relu(s_i + n_j, 0.2)
    alpha1 = softmax(E_re + NEG_BIG*(1-A)) ; alpha2 = softmax(E_im)
    out_re = alpha1 @ X_re - alpha2 @ X_im ; out_im = alpha1 @ X_im + alpha2 @ X_re

Sharding: 8 cores; core c handles batch b=c//4, query-row block rb=c%4 (1024 rows).
Attention tiles live in key-major [j, i] layout on chip.

v3 design:
  - flip-orientation contraction: for each key chunk the attention tile's
    128-query blocks are the matmul's stationary weights and [Xcat | 1] is the
    moving tensor (FD=129). Output lands as [query, channel] in PSUM with the
    softmax denominator as channel 128 -- row-sum matmuls and all epilogue
    transposes disappear. Probe-measured: 91 ns per FD=129 matmul,
    back-to-back, weight loads hidden.
  - 16 accumulators (8 query blocks x {u,v}) are hand-packed 3-per-PSUM-bank
    as 129-column slices of [128,512] bank tiles; the first slice written in
    each bank uses start=True (marks the whole 2KB zero-region), later slices
    rely on the pending-zero init with start=False.
  - im path uses exp(lrelu(s+n)) = max(exp(s)exp(n), exp(.2s)exp(.2n)):
    one fused DVE op (VMAX2) per chunk replaces lrelu+exp; a tunable subset of
    chunk-pairs instead goes through ScalarE (prelu+exp) to balance engines.
  - re path keeps mask-additive LRELU3 on DVE + exp on ScalarE (the mask
    tensor forces a third input stream, which only LRELU3 fuses).
  - everything runs fp16 (PE at 1 cyc/row; fp32 would be 2 instructions at
    4 cyc/row); exp folds a 1/2 scale in via bias to keep f16 headroom.
"""

import sys

if "/opt/trn_rl_repo" not in sys.path:
    sys.path.insert(0, "/opt/trn_rl_repo")

import numpy as np

import concourse.bass as bass
import concourse.tile as tile
from concourse import bacc, mybir
from concourse.bass_utils import run_bass_kernel_spmd

B, N, F, U = 2, 4096, 128, 64
NCORES = 8
ROWS = N * B // NCORES  # 1024 query rows per core
NCHUNK = N // 128  # 32 key chunks of 128
NPAIR = NCHUNK // 2
NBLK = ROWS // 128  # 8 query blocks per core
SLOPE = 0.2
# chunk-pairs whose im-path runs on ScalarE (prelu+exp) instead of DVE VMAX2
ACT_IM_PAIRS = set(range(1, 16, 3))  # 5 pairs -> 10 chunks
# chunk-pairs whose im-path max runs on GpSimd (DVE makes the two products)
POOL_IM_PAIRS = set()  # gpsimd tensor ops rejected by walrus codegen
MASKV = 448.0  # additive mask magnitude; exp underflows to exactly 0
MLN2 = -0.6931471805599453  # exp bias: weights scaled by 1/2 for f16 headroom
A_ = mybir.AluOpType
AF = mybir.ActivationFunctionType
f32 = mybir.dt.float32
f16 = mybir.dt.float16

# ---------------------------------------------------------------- custom DVE ops


def _register_ops():
    """Register the fused ops in dve_ops.OPS (idempotent)."""
    import concourse.dve_ops as dve_ops_mod
    from concourse.dve_ops import OPS, DveOp, _CUSTOM_DVE_ROW_BASE, _SUB_OPCODE_FOR_NAME
    from concourse.dve_spec import Spec, Src0, Src1, C0, C1, maxx, lower, _has_src1
    from concourse.dve_uop import DveOpSpec

    existing = {op.name: op for op in OPS}
    if "VMAX2_ANT" in existing:
        return (
            existing["LRELU_ADD3_ANT"],
            existing["VMAX2_ANT"],
        )

    def make(name, spec):
        row = _CUSTOM_DVE_ROW_BASE + len(OPS)
        _SUB_OPCODE_FOR_NAME[name] = row
        shas = {}
        for ver in ("v3", "v4"):
            tmp = DveOpSpec(
                name=name, opcode=row, uops=lower(spec, ver=ver), rd1_en=_has_src1(spec)
            )
            shas[ver] = tmp.sha(ver)
        op = DveOp(name, spec, subdim=False, uops_sha=shas)
        OPS.append(op)
        dve_ops_mod.CUSTOM_DVE_SPECS[name] = spec
        return op

    # out = lrelu(in0 + in1 + s0) with slope s1:  t = Src0+Src1+C0; max(t, t*C1)
    t3 = (Src0 + Src1) + C0
    spec3 = Spec(
        body=maxx(t3, t3 * C1),
        reference=lambda in0, in1, s0, s1, imm2: np.maximum(
            (in0 + in1 + s0), (in0 + in1 + s0) * s1
        ),
    )
    lrelu3 = make("LRELU_ADD3_ANT", spec3)
    # out = max(in0*s0, in1*s1)   (separable exp(lrelu(s+n)) for the im part)
    spec_vmax = Spec(
        body=maxx(Src0 * C0, Src1 * C1),
        reference=lambda in0, in1, s0, s1, imm2: np.maximum(in0 * s0, in1 * s1),
    )
    vmax2 = make("VMAX2_ANT", spec_vmax)
    return lrelu3, vmax2


# ---------------------------------------------------------------- device program

_PROGRAM_CACHE = {}

# psum slot map: (comp, ib) -> (bank_tile, slot) with 3 slots of 129 per bank
def _slot(comp, ib):
    idx = comp * NBLK + ib  # 0..15
    return idx // 3, idx % 3


def _build_program():
    if "nc" in _PROGRAM_CACHE:
        return _PROGRAM_CACHE["nc"]
    LRELU3, VMAX2 = _register_ops()

    nc = bacc.Bacc("TRN2", target_bir_lowering=False, debug=False, num_devices=NCORES)
    from concourse.tile_rust import add_dep_helper

    _pe_prev = [None]

    def mm(out, lhsT, rhs, reuse=False, **kw):
        bi = nc.tensor.matmul(out, lhsT=lhsT, rhs=rhs, **kw)
        if reuse:
            bi.ins.ldweights = False
        if _pe_prev[0] is not None:
            add_dep_helper(bi.ins, _pe_prev[0], sync=False, reason="pe order")
        _pe_prev[0] = bi.ins
        return bi

    dp = nc.dram_tensor
    ht_in = {
        "re": dp("ht_re", [F, N], f16, kind="ExternalInput").ap(),
        "im": dp("ht_im", [F, N], f16, kind="ExternalInput").ap(),
    }
    w_in = dp("w", [F, U], f16, kind="ExternalInput").ap()
    wt_in = dp("wt", [U, F], f16, kind="ExternalInput").ap()
    acat_in = dp("acat", [U, 2], f16, kind="ExternalInput").ap()
    mt_in = dp("mt", [N, ROWS], f16, kind="ExternalInput").ap()
    o_re = dp("o_re", [ROWS, U], f32, kind="ExternalOutput").ap()
    o_im = dp("o_im", [ROWS, U], f32, kind="ExternalOutput").ap()

    with tile.TileContext(nc) as tc:
        with tc.tile_pool(name="cst", bufs=1) as cst:
            # ---- small constants
            w_sb = cst.tile([F, U], f16, tag="w", name="w")
            nc.sync.dma_start(w_sb[:], w_in[:])
            wt_sb = cst.tile([U, F], f16, tag="wt", name="wt")
            nc.sync.dma_start(wt_sb[:], wt_in[:])
            acat_sb = cst.tile([U, 2], f16, tag="acat", name="acat")
            nc.sync.dma_start(acat_sb[:], acat_in[:])
            mln2_sb = cst.tile([128, 1], f32, tag="mln2", name="mln2")
            nc.gpsimd.memset(mln2_sb[:], MLN2)

            # ---- ht loaded up front in group-sized pieces (overlaps X-gen)
            ht_sb = {}
            for nm in ("re", "im"):
                t = cst.tile([F, N], f16, tag=f"ht_{nm}", name=f"ht_{nm}")
                ht_sb[nm] = t
            for g in range(4):
                for nm in ("re", "im"):
                    lo = ROWS * g
                    nc.gpsimd.dma_start(
                        ht_sb[nm][:, lo : lo + ROWS], ht_in[nm][:, lo : lo + ROWS]
                    )

            # [Xre | Xim | 1 | pad] per chunk, 130 columns (128..129 preset to 1)
            xcp_sb = cst.tile([128, NCHUNK * 130], f16, tag="xcp", name="xcp")
            nc.gpsimd.memset(xcp_sb[:], 1.0)
            xcp3 = xcp_sb[:].rearrange("p (c u) -> p c u", u=130)
            sn_sb = {
                nm: cst.tile([128, 2 * NCHUNK], f32, tag=f"sn_{nm}", name=f"sn_{nm}")
                for nm in ("re", "im")
            }
            sbc_sb = {
                nm: cst.tile([128, ROWS], f16, tag=f"sbc_{nm}", name=f"sbc_{nm}")
                for nm in ("re", "im")
            }
            # exp(n_im)/1 and exp(.2 n_im) per key, chunk-major [128, NCHUNK]
            c_sb = cst.tile([128, NCHUNK], f32, tag="c", name="c")
            d_sb = cst.tile([128, NCHUNK], f32, tag="d", name="d")
            # exp(s_im)/2, exp(.2 s_im)/2 broadcast over partitions [128, ROWS]
            pb_sb = cst.tile([128, ROWS], f16, tag="pb", name="pb")
            qb_sb = cst.tile([128, ROWS], f16, tag="qb", name="qb")

            # ---- phase b: wa, then s-broadcast in one matmul hop via
            #      W1[F, p] = wa[F, 0] (rank-1 weights -> out[p, i] = s_i for all p)
            ones128_sb = cst.tile([128, 128], f16, tag="ones128", name="ones128")
            nc.gpsimd.memset(ones128_sb[:], 1.0)
            with tc.tile_pool(name="psB", bufs=1, space="PSUM") as psB:
                wa_ps = psB.tile([F, 2], f32, tag="wa_ps", name="wa_ps")
                mm(wa_ps[:], wt_sb[:], acat_sb[:], start=True, stop=True)
                wa16_sb = cst.tile([F, 2], f16, tag="wa16", name="wa16")
                nc.vector.tensor_copy(wa16_sb[:], wa_ps[:])
                w1_sb = cst.tile([128, 128], f16, tag="w1b", name="w1b")
                nc.vector.tensor_scalar_mul(w1_sb[:], ones128_sb[:], wa_ps[:, 0:1])
                for nm in ("re", "im"):
                    sb_ps = psB.tile([128, ROWS], f32, tag="sb_ps", name="sb_ps")
                    for h in range(2):
                        mm(
                            sb_ps[:, 512 * h : 512 * (h + 1)],
                            w1_sb[:],
                            ht_sb[nm][:, 512 * h : 512 * (h + 1)],
                            start=True,
                            stop=True,
                        )
                    nc.scalar.copy(sbc_sb[nm][:], sb_ps[:])
                # pb/qb from the already-broadcast sbc_im (1/2 scale folded in)
                nc.scalar.activation(pb_sb[:], sbc_sb["im"][:], AF.Exp, bias=mln2_sb[:])
                nc.scalar.activation(
                    qb_sb[:], sbc_sb["im"][:], AF.Exp, bias=mln2_sb[:], scale=SLOPE
                )

            # ---- interleaved groups: X/sn generation + attention chunks
            with (
                tc.tile_pool(name="psG", bufs=1, space="PSUM") as psG,
                tc.tile_pool(name="psM", bufs=1, space="PSUM") as psM,
                tc.tile_pool(name="mtp", bufs=6) as mt_pool,
                tc.tile_pool(name="rp", bufs=3) as r_pool,
                tc.tile_pool(name="uvp", bufs=3) as uv_pool,
                tc.tile_pool(name="tscp", bufs=3) as tsc_pool,
            ):
                # 6 PSUM bank tiles, each holding 3 slices of 129 columns
                pbank = [
                    psM.tile([128, 512], f32, tag=f"pb{i}", name=f"pbank{i}")
                    for i in range(6)
                ]

                def pslot(comp, ib):
                    bt, sl = _slot(comp, ib)
                    return pbank[bt][:, 129 * sl : 129 * sl + 129]

                started_banks = set()
                nim = sn_sb["im"][:].rearrange("p (c two) -> p c two", two=2)

                def xgen(g):
                    """X/s/n generation + c/d for group g (8 chunks)."""
                    for nm in ("re", "im"):
                        sn_ps = psG.tile([128, 16], f32, tag="sn_ps", name="sn_ps")
                        x_ps = psG.tile([128, 512], f32, tag="x_ps", name="x_ps")
                        for m in range(8):
                            k = 8 * g + m
                            hchunk = ht_sb[nm][:, 128 * k : 128 * (k + 1)]
                            mm(
                                x_ps[:, 64 * m : 64 * (m + 1)],
                                hchunk,
                                w_sb[:],
                                start=True,
                                stop=True,
                            )
                        for m in range(8):
                            k = 8 * g + m
                            hchunk = ht_sb[nm][:, 128 * k : 128 * (k + 1)]
                            mm(
                                sn_ps[:, 2 * m : 2 * m + 2],
                                hchunk,
                                wa16_sb[:],
                                start=True,
                                stop=True,
                            )
                        off = 0 if nm == "re" else 64
                        nc.scalar.copy(
                            xcp3[:, 8 * g : 8 * g + 8, off : off + 64],
                            x_ps[:].rearrange("p (c u) -> p c u", u=64),
                        )
                        nc.vector.tensor_copy(
                            sn_sb[nm][:, 16 * g : 16 * g + 16], sn_ps[:]
                        )
                    nc.scalar.activation(
                        c_sb[:, 8 * g : 8 * g + 8], nim[:, 8 * g : 8 * g + 8, 1],
                        AF.Exp,
                    )
                    nc.scalar.activation(
                        d_sb[:, 8 * g : 8 * g + 8], nim[:, 8 * g : 8 * g + 8, 1],
                        AF.Exp, scale=SLOPE,
                    )

                xgen(0)
                xgen(1)

                # ---- attention pairs, with group g+2's X-gen interleaved
                for g in range(4):
                    if g + 2 < 4:
                        xgen(g + 2)
                    for t2i in range(4):
                        pr = 4 * g + t2i
                        ka = 2 * pr
                        act_im = pr in ACT_IM_PAIRS
                        rw = r_pool.tile([128, 4 * ROWS], f16, tag="rw", name="rw")
                        uvw = uv_pool.tile([128, 4 * ROWS], f16, tag="uv", name="uvw")
                        for c2 in range(2):
                            k = ka + c2
                            mt_t = mt_pool.tile([128, ROWS], f16, tag="mt", name="mt")
                            nc.sync.dma_start(
                                mt_t[:], mt_in[128 * k : 128 * (k + 1), :]
                            )
                            nc.vector._custom_dve(
                                LRELU3,
                                out=rw[:, ROWS * c2 : ROWS * (c2 + 1)],
                                in0=mt_t[:],
                                in1=sbc_sb["re"][:],
                                s0=sn_sb["re"][:, 2 * k + 1 : 2 * k + 2],
                                s1=SLOPE,
                            )
                            if act_im:
                                nc.scalar.activation(
                                    rw[:, ROWS * (2 + c2) : ROWS * (3 + c2)],
                                    sbc_sb["im"][:],
                                    AF.Prelu,
                                    bias=sn_sb["im"][:, 2 * k + 1 : 2 * k + 2],
                                    alpha=SLOPE,
                                )
                            elif pr in POOL_IM_PAIRS:
                                t1 = tsc_pool.tile(
                                    [128, ROWS], f16, tag="t1", name="t1"
                                )
                                t2 = tsc_pool.tile(
                                    [128, ROWS], f16, tag="t2", name="t2"
                                )
                                nc.vector.tensor_scalar(
                                    t1[:], pb_sb[:], c_sb[:, k : k + 1], None,
                                    A_.mult,
                                )
                                nc.vector.tensor_scalar(
                                    t2[:], qb_sb[:], d_sb[:, k : k + 1], None,
                                    A_.mult,
                                )
                                nc.gpsimd.tensor_max(
                                    uvw[:, ROWS * (2 + c2) : ROWS * (3 + c2)],
                                    t1[:],
                                    t2[:],
                                )
                            else:
                                nc.vector._custom_dve(
                                    VMAX2,
                                    out=uvw[:, ROWS * (2 + c2) : ROWS * (3 + c2)],
                                    in0=pb_sb[:],
                                    in1=qb_sb[:],
                                    s0=c_sb[:, k : k + 1],
                                    s1=d_sb[:, k : k + 1],
                                )
                        if act_im:
                            # one wide exp covers re pair + im pair
                            nc.scalar.activation(
                                uvw[:, 0 : 4 * ROWS], rw[:, 0 : 4 * ROWS], AF.Exp,
                                bias=mln2_sb[:],
                            )
                        else:
                            nc.scalar.activation(
                                uvw[:, 0 : 2 * ROWS], rw[:, 0 : 2 * ROWS], AF.Exp,
                                bias=mln2_sb[:],
                            )
                        # flip-orientation matmuls: attention blocks are the
                        # stationary weights; [Xcat | 1] streams through.
                        # (comp, ib) emitted slot-major so consecutive matmuls
                        # hit different PSUM banks and slot-0 starts come first.
                        order = sorted(
                            ((comp, ib) for comp in range(2) for ib in range(NBLK)),
                            key=lambda ci: (_slot(ci[0], ci[1])[1], _slot(ci[0], ci[1])[0]),
                        )
                        for c2 in range(2):
                            k = ka + c2
                            rhs = xcp3[:, k, 0:129]
                            for comp, ib in order:
                                base = ROWS * (2 * comp + c2)
                                out_ap = pslot(comp, ib)
                                bt, sl = _slot(comp, ib)
                                if k == 0:
                                    st = bt not in started_banks
                                    started_banks.add(bt)
                                else:
                                    st = False
                                mm(
                                    out_ap,
                                    uvw[:, base + 128 * ib : base + 128 * (ib + 1)],
                                    rhs,
                                    start=st,
                                    stop=(k == NCHUNK - 1),
                                    skip_group_check=True,
                                )

                # ---- epilogue: normalize and combine directly from PSUM
                with tc.tile_pool(name="ep2", bufs=8) as ep2:
                    ore_all = ep2.tile([128, NBLK * U], f32, tag="orea", name="orea")
                    oim_all = ep2.tile([128, NBLK * U], f32, tag="oima", name="oima")
                    for ib in range(NBLK):
                        pu = pslot(0, ib)
                        pv = pslot(1, ib)
                        rr = ep2.tile([128, 2], f32, tag="rr", name="rr")
                        nc.vector.reciprocal(rr[:, 0:1], pu[:, 128:129])
                        nc.vector.reciprocal(rr[:, 1:2], pv[:, 128:129])
                        # out_re = pu[:, :64]/su - pv[:, 64:]/sv
                        w1 = ep2.tile([128, U], f32, tag="w1", name="w1")
                        nc.scalar.activation(
                            w1[:], pv[:, U : 2 * U], AF.Copy, scale=rr[:, 1:2]
                        )
                        nc.vector.scalar_tensor_tensor(
                            ore_all[:, U * ib : U * (ib + 1)],
                            pu[:, 0:U],
                            rr[:, 0:1],
                            w1[:],
                            A_.mult,
                            A_.subtract,
                        )
                        # out_im = pu[:, 64:]/su + pv[:, :64]/sv
                        w2 = ep2.tile([128, U], f32, tag="w2", name="w2")
                        nc.scalar.activation(
                            w2[:], pv[:, 0:U], AF.Copy, scale=rr[:, 1:2]
                        )
                        nc.vector.scalar_tensor_tensor(
                            oim_all[:, U * ib : U * (ib + 1)],
                            pu[:, U : 2 * U],
                            rr[:, 0:1],
                            w2[:],
                            A_.mult,
                            A_.add,
                        )
                    # single batched DMA per output: dram [1024, U] viewed as
                    # [128 partitions, NBLK*U] with row = 128*ib + p
                    o_re_v = o_re.rearrange("(c p) u -> p c u", p=128)
                    o_im_v = o_im.rearrange("(c p) u -> p c u", p=128)
                    nc.sync.dma_start(
                        o_re_v, ore_all[:].rearrange("p (c u) -> p c u", u=U)
                    )
                    nc.sync.dma_start(
                        o_im_v, oim_all[:].rearrange("p (c u) -> p c u", u=U)
                    )

    nc.compile()
    _PROGRAM_CACHE["nc"] = nc
    return nc


# ---------------------------------------------------------------- host wrapper


def _make_in_maps(H_re, H_im, A, W, a_1, a_2):
    W16 = np.asarray(W, np.float16)
    acat = np.concatenate(
        [np.asarray(a_1, np.float16), np.asarray(a_2, np.float16)], axis=1
    )
    shared = {
        "w": W16,
        "wt": np.ascontiguousarray(W16.T),
        "acat": acat,
    }
    in_maps = []
    for c in range(NCORES):
        b, rb = divmod(c, NCORES // B)
        r0 = rb * ROWS
        hre = np.asarray(H_re[b], np.float16)
        him = np.asarray(H_im[b], np.float16)
        ab = np.asarray(A[b], np.float32)
        # key order rolled so this core's own query rows come first
        mt = np.ascontiguousarray(
            ((np.roll(ab[r0 : r0 + ROWS].T, -r0, axis=0) - 1.0) * MASKV).astype(
                np.float16
            )
        )
        in_maps.append(
            {
                **shared,
                "ht_re": np.ascontiguousarray(np.roll(hre, -r0, axis=0).T),
                "ht_im": np.ascontiguousarray(np.roll(him, -r0, axis=0).T),
                "mt": mt,
            }
        )
    return in_maps


def kernel(H_re, H_im, A, W, a_1, a_2):
    nc = _build_program()
    in_maps = _make_in_maps(H_re, H_im, A, W, a_1, a_2)
    res = run_bass_kernel_spmd(nc, in_maps, list(range(NCORES)))
    out_re = np.empty((B, N, U), np.float32)
    out_im = np.empty((B, N, U), np.float32)
    for c in range(NCORES):
        b, rb = divmod(c, NCORES // B)
        r0 = rb * ROWS
        out_re[b, r0 : r0 + ROWS] = res.results[c]["o_re"]
        out_im[b, r0 : r0 + ROWS] = res.results[c]["o_im"]
    return out_re, out_im
